# revision 13
# baseline (speedup 1.0000x reference)
"""Trainium2 Bass kernel for nn_H_H_EdgeApplyModule (GNN edge-apply).

Reference computation:
    feat      = concat([n_f[src], s_f, n_f[dst]], 1)          # [E, 3072]
    feat_lang = concat([word2vec[src], word2vec[dst]], 1)     # [E, 600]
    e_f       = relu(feat @ W1 + b1)                          # [E, 256]
    e_f_lang  = relu(feat_lang @ Wl + bl)                     # [E, 256]

Algebraic restructure (cuts FLOPs 2.7x and gather bytes 2.4x):
    W1 = [W1a; W1b; W1c] (rows 0:1024, 1024:2048, 2048:3072)
    Wl = [Wla; Wlb]      (rows 0:300, 300:600)
    P  = n_f @ W1a + b1   Q  = n_f @ W1c
    Pl = w2v @ Wla + bl   Ql = w2v @ Wlb
    e_f      = relu(P[src] + s_f @ W1b + Q[dst])
    e_f_lang = relu(Pl[src] + Ql[dst])

Distribution (8 cores):
    - Node tables: each core computes a 1/8 shard of the combined table
      T = [[P|Pl]; [Q|Ql]] (4096 rows x 512), then ONE AllGather of the
      merged buffer -> full 32768x512 f16 table in local DRAM (a single
      32MiB collective hits full fabric bandwidth; two 16MiB ones do not).
      Gather row ids are remapped on the host to the AllGather layout.
    - Edges: sharded contiguously; each core handles E/8 edges.
    - Phase 2 is split in two decoupled stages so the s_f GEMM overlaps
      the collective:
        stage A (independent of collective): sfW = s_f @ W1b -> SBUF f16
        stage B (after collective): dma_gather rows, DVE adds, relu,
        f16 outputs (host upcasts to f32).
    - All activations arrive pre-transposed/padded f16 from the host
      (lhsT layout), eliminating on-device PE transposes and casts.
"""

import sys

sys.path.insert(0, "/opt/trn_rl_repo")

import numpy as np

from concourse import bass, bacc, tile, mybir
from concourse.bass_utils import run_bass_kernel_spmd

F32 = mybir.dt.float32
F16 = mybir.dt.float16
I16 = mybir.dt.int16

# ---------------------------------------------------------------- config
N_CORES = 8
N_NODES = 16384
E_TOTAL = 131072
D = 1024          # node/spatial feature dim
DW_PAD = 384      # word2vec dim padded 300 -> 384 (3 full 128-chunks)
DOUT = 256
TBL = 512         # table row: [P|Pl] or [Q|Ql]

E_CORE = E_TOTAL // N_CORES          # 16384
NODE_SHARD = N_NODES // N_CORES      # 2048
EDGE_TILE = 128
BATCH_TILES = 8                      # edge tiles per gather batch
BATCH = EDGE_TILE * BATCH_TILES      # 1024 edges per gather
KC_D = D // 128                      # 8 K-chunks for 1024-dim features
KC_W = DW_PAD // 128                 # 3 K-chunks for word2vec


def build_kernel(n_cores=N_CORES, node_shard=NODE_SHARD, e_core=E_CORE,
                 batch_tiles=BATCH_TILES):
    n_nodes = node_shard * n_cores
    batch = EDGE_TILE * batch_tiles
    n_batches = e_core // batch
    node_tiles = node_shard // 128
    edge_tiles = e_core // EDGE_TILE
    idx_cols = e_core // 16

    nc = bacc.Bacc("TRN2", target_bir_lowering=False, debug=False,
                   num_devices=n_cores)

    # ---------------- I/O ----------------
    # activations pre-transposed on host to lhsT layout, f16:
    #   nfT[t] is [128 K-partitions, KC_D*128 free] for node tile t
    nfT = nc.declare_dram_parameter("nfT", [node_tiles, 128, KC_D * 128], F16,
                                    isOutput=False)
    w2vT = nc.declare_dram_parameter("w2vT", [node_tiles, 128, KC_W * 128], F16,
                                     isOutput=False)
    sfT = nc.declare_dram_parameter("sfT", [edge_tiles, 128, KC_D * 128], F16,
                                    isOutput=False)
    w_nf = nc.declare_dram_parameter("w_nf", [D, TBL], F16, isOutput=False)       # [W1a|W1c]
    w_l = nc.declare_dram_parameter("w_l", [DW_PAD, TBL], F16, isOutput=False)    # [Wla|Wlb]
    w1b = nc.declare_dram_parameter("w1b", [D, DOUT], F16, isOutput=False)
    bias = nc.declare_dram_parameter("bias_src", [1, TBL], F32, isOutput=False)   # [b1|bl]
    ones = nc.declare_dram_parameter("ones", [1, 128], F32, isOutput=False)
    idx_src = nc.declare_dram_parameter("idx_src", [128, idx_cols], I16, isOutput=False)
    idx_dst = nc.declare_dram_parameter("idx_dst", [128, idx_cols], I16, isOutput=False)
    out_e = nc.declare_dram_parameter("out_e", [e_core, DOUT], F16, isOutput=True)
    out_l = nc.declare_dram_parameter("out_l", [e_core, DOUT], F16, isOutput=True)

    # ---------------- internal DRAM ----------------
    # merged table: rows [0:shard]=Tsrc shard, [shard:2*shard]=Tdst shard
    tbl_sh = nc.dram_tensor("tbl_shard", [2 * node_shard, TBL], F16)
    tbl_full = nc.dram_tensor("tbl_full", [2 * n_nodes, TBL], F16,
                              addr_space="Shared")

    with tile.TileContext(nc) as tc:
        with (
            tc.tile_pool(name="const", bufs=1) as cpool,
            tc.tile_pool(name="psum_b", bufs=1, space="PSUM") as pbias,
            tc.tile_pool(name="sfw", bufs=1) as sfwpool,
        ):
            # persistent constants in SBUF
            w_nf_sb = cpool.tile([128, KC_D, TBL], F16)
            nc.sync.dma_start(w_nf_sb[:], w_nf[:].rearrange("(c p) n -> p c n", p=128))
            w_l_sb = cpool.tile([128, KC_W, TBL], F16)
            nc.sync.dma_start(w_l_sb[:], w_l[:].rearrange("(c p) n -> p c n", p=128))
            w1b_sb = cpool.tile([128, KC_D, DOUT], F16)
            nc.sync.dma_start(w1b_sb[:], w1b[:].rearrange("(c p) n -> p c n", p=128))
            ones_sb = cpool.tile([1, 128], F32)
            nc.sync.dma_start(ones_sb[:], ones[:])
            bias_sb = cpool.tile([1, TBL], F32)
            nc.sync.dma_start(bias_sb[:], bias[:])
            idx_src_sb = cpool.tile([128, idx_cols], I16)
            nc.sync.dma_start(idx_src_sb[:], idx_src[:])
            idx_dst_sb = cpool.tile([128, idx_cols], I16)
            nc.sync.dma_start(idx_dst_sb[:], idx_dst[:])

            # broadcast bias to all 128 partitions: psum = ones.T @ bias
            bias_full = cpool.tile([128, TBL], F32)
            pb = pbias.tile([128, TBL], F32)
            nc.tensor.matmul(pb[:], ones_sb[:], bias_sb[:], start=True, stop=True)
            nc.vector.tensor_copy(bias_full[:], pb[:])

            # stage-A result buffer: sfW[t] = (s_f @ W1b) tile t, f16
            sfw_sb = sfwpool.tile([128, edge_tiles, DOUT], F16)

            # ============ phase 1: node-table shard ============
            with (
                tc.tile_pool(name="p1_in", bufs=2) as p1in,
                tc.tile_pool(name="p1_out", bufs=2) as p1out,
                tc.tile_pool(name="p1_psrc", bufs=2, space="PSUM") as p1psrc,
                tc.tile_pool(name="p1_pdst", bufs=2, space="PSUM") as p1pdst,
            ):
                for nt in range(node_tiles):
                    r0 = nt * 128
                    nfT_t = p1in.tile([128, KC_D, 128], F16, tag="nfT")
                    nc.sync.dma_start(nfT_t[:], nfT[nt])
                    w2vT_t = p1in.tile([128, KC_W, 128], F16, tag="w2vT")
                    nc.sync.dma_start(w2vT_t[:], w2vT[nt])

                    # Tsrc = [P | Pl] + [b1|bl],  Tdst = [Q | Ql]
                    ps = p1psrc.tile([128, TBL], F32)
                    pd = p1pdst.tile([128, TBL], F32)
                    for kc in range(KC_D):
                        nc.tensor.matmul(
                            ps[:, 0:DOUT], nfT_t[:, kc, :], w_nf_sb[:, kc, 0:DOUT],
                            start=(kc == 0), stop=(kc == KC_D - 1))
                    for kc in range(KC_W):
                        nc.tensor.matmul(
                            ps[:, DOUT:TBL], w2vT_t[:, kc, :], w_l_sb[:, kc, 0:DOUT],
                            start=(kc == 0), stop=(kc == KC_W - 1))
                    for kc in range(KC_D):
                        nc.tensor.matmul(
                            pd[:, 0:DOUT], nfT_t[:, kc, :], w_nf_sb[:, kc, DOUT:TBL],
                            start=(kc == 0), stop=(kc == KC_D - 1))
                    for kc in range(KC_W):
                        nc.tensor.matmul(
                            pd[:, DOUT:TBL], w2vT_t[:, kc, :], w_l_sb[:, kc, DOUT:TBL],
                            start=(kc == 0), stop=(kc == KC_W - 1))

                    src_o = p1out.tile([128, TBL], F16, tag="src_o")
                    dst_o = p1out.tile([128, TBL], F16, tag="dst_o")
                    nc.vector.tensor_add(src_o[:], ps[:], bias_full[:])
                    nc.scalar.copy(dst_o[:], pd[:])
                    nc.sync.dma_start(tbl_sh[r0:r0 + 128, :], src_o[:])
                    nc.sync.dma_start(tbl_sh[node_shard + r0:node_shard + r0 + 128, :],
                                      dst_o[:])

            # ============ single AllGather of the merged table ============
            groups = [list(range(n_cores))]
            nc.gpsimd.collective_compute(
                "AllGather", mybir.AluOpType.bypass, replica_groups=groups,
                ins=[tbl_sh[:]], outs=[tbl_full[:]])

            # ============ phase 2, stage A: sfW = s_f @ W1b ============
            # (independent of the collective; overlaps it)
            with (
                tc.tile_pool(name="p2_sfT", bufs=4) as p2sft,
                tc.tile_pool(name="p2_pa", bufs=6, space="PSUM") as p2pa,
            ):
                for t0 in range(0, edge_tiles, 2):
                    sfT_t = p2sft.tile([128, 2, KC_D, 128], F16, tag="sfT")
                    nc.sync.dma_start(
                        sfT_t[:], sfT[t0:t0 + 2].rearrange("t p f -> p t f"))
                    pa = p2pa.tile([128, 2, DOUT], F32)
                    for tt in range(2):
                        for kc in range(KC_D):
                            nc.tensor.matmul(
                                pa[:, tt, :], sfT_t[:, tt, kc, :], w1b_sb[:, kc, :],
                                start=(kc == 0), stop=(kc == KC_D - 1))
                    nc.scalar.copy(sfw_sb[:, t0:t0 + 2, :], pa[:])

            # ============ phase 2, stage B: gather + add + relu + out =====
            # Opened AFTER the stage-A pools close: the stack allocator
            # reuses stage A's SBUF zone, so every stage-B tile carries a
            # dependency on stage-A completion.  That keeps the scheduler
            # from interleaving stage-B ops (which stall on the collective)
            # ahead of stage-A ops on shared engine queues.
            GRP = 4                       # edge tiles per stage-B group
            with (
                tc.tile_pool(name="p2_g", bufs=4) as p2g,
                tc.tile_pool(name="p2_sum", bufs=4) as p2sum,
                tc.tile_pool(name="p2_out", bufs=4) as p2out,
            ):
                for b in range(n_batches):
                    c0 = b * (batch // 16)
                    g_src = p2g.tile([128, batch_tiles, TBL], F16, tag="gs")
                    nc.gpsimd.dma_gather(
                        g_src[:], tbl_full[:], idx_src_sb[:, c0:c0 + batch // 16],
                        batch, batch, TBL)
                    g_dst = p2g.tile([128, batch_tiles, TBL], F16, tag="gd")
                    nc.gpsimd.dma_gather(
                        g_dst[:], tbl_full[:], idx_dst_sb[:, c0:c0 + batch // 16],
                        batch, batch, TBL)

                    for t in range(0, batch_tiles, GRP):
                        at = b * batch_tiles + t
                        e0 = at * EDGE_TILE
                        tsum = p2sum.tile([128, GRP, TBL], F16, tag="tsum")
                        nc.vector.tensor_add(tsum[:], g_src[:, t:t + GRP, :],
                                             g_dst[:, t:t + GRP, :])
                        oe = p2out.tile([128, GRP, DOUT], F16, tag="oe")
                        ol = p2out.tile([128, GRP, DOUT], F16, tag="ol")
                        esum = p2sum.tile([128, GRP, DOUT], F16, tag="esum")
                        nc.vector.tensor_add(esum[:], tsum[:, :, 0:DOUT],
                                             sfw_sb[:, at:at + GRP, :])
                        nc.scalar.activation(
                            oe[:], esum[:], mybir.ActivationFunctionType.Relu)
                        nc.vector.tensor_scalar_max(
                            ol[:], tsum[:, :, DOUT:TBL], 0.0)
                        nc.scalar.dma_start(
                            out_e[e0:e0 + GRP * 128, :].rearrange(
                                "(t p) n -> p t n", p=128), oe[:])
                        nc.sync.dma_start(
                            out_l[e0:e0 + GRP * 128, :].rearrange(
                                "(t p) n -> p t n", p=128), ol[:])

    nc.compile()
    return nc


# ---------------------------------------------------------------- host side
def _wrap_idx(ix, batch):
    """int16 index layout for dma_gather: idx j of a batch sits at
    (partition j%16, column j//16); 16-row block replicated to 128."""
    e = ix.shape[0]
    n_b = e // batch
    cols = batch // 16
    arr = np.zeros((16, e // 16), dtype=np.int16)
    for b in range(n_b):
        blk = ix[b * batch:(b + 1) * batch].astype(np.int16).reshape(cols, 16).T
        arr[:, b * cols:(b + 1) * cols] = blk
    return np.ascontiguousarray(np.tile(arr, (8, 1)))


def _to_lhsT(x16, kc):
    """[rows, kc*128] f16 -> [rows/128 tiles, 128 K-partitions, kc*128] lhsT."""
    tiles = x16.shape[0] // 128
    return np.ascontiguousarray(
        x16.reshape(tiles, 128, kc, 128).transpose(0, 3, 2, 1)
        .reshape(tiles, 128, kc * 128))


_NC_CACHE = {}


def make_in_maps(n_f, word2vec, s_f, W1, b1, Wl, bl, src, dst):
    n_f = np.asarray(n_f, dtype=np.float32)
    word2vec = np.asarray(word2vec, dtype=np.float32)
    s_f = np.asarray(s_f, dtype=np.float32)
    W1 = np.asarray(W1, dtype=np.float32)
    Wl = np.asarray(Wl, dtype=np.float32)
    b1 = np.asarray(b1, dtype=np.float32)
    bl = np.asarray(bl, dtype=np.float32)
    src = np.asarray(src).astype(np.int64)
    dst = np.asarray(dst).astype(np.int64)

    w2v_pad = np.zeros((N_NODES, DW_PAD), np.float16)
    w2v_pad[:, :300] = word2vec.astype(np.float16)
    n_f16 = n_f.astype(np.float16)
    s_f16 = s_f.astype(np.float16)

    w_nf = np.ascontiguousarray(
        np.concatenate([W1[0:D], W1[2 * D:3 * D]], axis=1)).astype(np.float16)
    w_l = np.zeros((DW_PAD, TBL), np.float16)
    w_l[:300, 0:DOUT] = Wl[0:300]
    w_l[:300, DOUT:TBL] = Wl[300:600]
    w1b = np.ascontiguousarray(W1[D:2 * D]).astype(np.float16)
    bias_src = np.concatenate([b1, bl])[None, :].astype(np.float32)
    ones = np.ones((1, 128), np.float32)

    # gather-row remap to the merged AllGather layout:
    #   node n (shard c=n//NODE_SHARD, local r): src row = 2*NODE_SHARD*c + r,
    #   dst row = src row + NODE_SHARD
    row_src = 2 * NODE_SHARD * (src // NODE_SHARD) + (src % NODE_SHARD)
    row_dst = 2 * NODE_SHARD * (dst // NODE_SHARD) + NODE_SHARD + (dst % NODE_SHARD)

    in_maps = []
    for k in range(N_CORES):
        es, ee = k * E_CORE, (k + 1) * E_CORE
        ns, ne = k * NODE_SHARD, (k + 1) * NODE_SHARD
        in_maps.append({
            "nfT": _to_lhsT(n_f16[ns:ne], KC_D),
            "w2vT": _to_lhsT(w2v_pad[ns:ne], KC_W),
            "sfT": _to_lhsT(s_f16[es:ee], KC_D),
            "w_nf": w_nf,
            "w_l": w_l,
            "w1b": w1b,
            "bias_src": bias_src,
            "ones": ones,
            "idx_src": _wrap_idx(row_src[es:ee], BATCH),
            "idx_dst": _wrap_idx(row_dst[es:ee], BATCH),
        })

    return in_maps


def kernel(n_f, word2vec, s_f, W1, b1, Wl, bl, src, dst):
    if "nc" not in _NC_CACHE:
        _NC_CACHE["nc"] = build_kernel()
    nc = _NC_CACHE["nc"]
    in_maps = make_in_maps(n_f, word2vec, s_f, W1, b1, Wl, bl, src, dst)
    res = run_bass_kernel_spmd(nc, in_maps, list(range(N_CORES)))
    _NC_CACHE["last_results"] = res
    e_f = np.concatenate(
        [res.results[k]["out_e"] for k in range(N_CORES)]).astype(np.float32)
    e_f_lang = np.concatenate(
        [res.results[k]["out_l"] for k in range(N_CORES)]).astype(np.float32)
    return (e_f, e_f_lang)


# revision 24
# speedup vs baseline: 2.9122x; 2.9122x over previous
"""Trainium2 Bass kernel for nn_H_H_EdgeApplyModule (GNN edge-apply).

Reference computation:
    feat      = concat([n_f[src], s_f, n_f[dst]], 1)          # [E, 3072]
    feat_lang = concat([word2vec[src], word2vec[dst]], 1)     # [E, 600]
    e_f       = relu(feat @ W1 + b1)                          # [E, 256]
    e_f_lang  = relu(feat_lang @ Wl + bl)                     # [E, 256]

Algebraic restructure (cuts FLOPs 2.7x and gather bytes 2.4x):
    W1 = [W1a; W1b; W1c] (rows 0:1024, 1024:2048, 2048:3072)
    Wl = [Wla; Wlb]      (rows 0:300, 300:600)
    P  = n_f @ W1a + b1   Q  = n_f @ W1c
    Pl = w2v @ Wla + bl   Ql = w2v @ Wlb
    e_f      = relu(P[src] + s_f @ W1b + Q[dst])
    e_f_lang = relu(Pl[src] + Ql[dst])

Distribution (8 cores):
    - Node tables: each core computes a 1/8 shard of Tsrc=[P|Pl] and
      Tdst=[Q|Ql] (f16), then two AllGathers -> full tables in local
      DRAM.  (Two 16MiB collectives measured faster on hardware than
      one merged 32MiB one, despite the cost model preferring merged.)
    - Edges: sharded contiguously; each core handles E/8 edges.
    - Phase 2 is split in two decoupled stages so the s_f GEMM overlaps
      the collective:
        stage A (independent of collective): sfW = s_f @ W1b -> SBUF f16
        stage B (after collective): dma_gather rows, DVE adds, relu,
        f16 outputs (host upcasts to f32).
    - All activations arrive pre-transposed/padded f16 from the host
      (lhsT layout), eliminating on-device PE transposes and casts.
    - All f16 operands travel in ONE packed DRAM buffer and both indices
      in one i16 buffer, outputs in one [2,E/8,256] buffer: the axon
      per-launch dispatch cost scales with the NUMBER of I/O buffers
      (~70us each), so 17 buffer slots -> 5 saves ~0.8ms per launch.
"""

import sys

sys.path.insert(0, "/opt/trn_rl_repo")

import numpy as np

from concourse import bass, bacc, tile, mybir
from concourse.bass_utils import run_bass_kernel_spmd

F32 = mybir.dt.float32
F16 = mybir.dt.float16
I16 = mybir.dt.int16

# ---------------------------------------------------------------- config
N_CORES = 8
N_NODES = 16384
E_TOTAL = 131072
D = 1024          # node/spatial feature dim
DW_PAD = 384      # word2vec dim padded 300 -> 384 (3 full 128-chunks)
DOUT = 256
TBL = 512         # table row: [P|Pl] or [Q|Ql]

E_CORE = E_TOTAL // N_CORES          # 16384
NODE_SHARD = N_NODES // N_CORES      # 2048
EDGE_TILE = 128
BATCH_TILES = 8                      # edge tiles per gather batch
BATCH = EDGE_TILE * BATCH_TILES      # 1024 edges per gather
KC_D = D // 128                      # 8 K-chunks for 1024-dim features
KC_W = DW_PAD // 128                 # 3 K-chunks for word2vec

NODE_TILES = NODE_SHARD // 128       # 16
EDGE_TILES = E_CORE // EDGE_TILE     # 128
IDX_COLS = E_CORE // 16              # 1024

# packed f16 input buffer layout (offsets in f16 elements)
SZ_NFT = NODE_TILES * 128 * KC_D * 128       # 2,097,152
SZ_W2VT = NODE_TILES * 128 * KC_W * 128      # 786,432
SZ_SFT = EDGE_TILES * 128 * KC_D * 128       # 16,777,216
SZ_WNF = D * TBL                             # 524,288
SZ_WL = DW_PAD * TBL                         # 196,608
SZ_W1B = D * DOUT                            # 262,144
SZ_BIAS = TBL                                # 512
OFF_NFT = 0
OFF_W2VT = OFF_NFT + SZ_NFT
OFF_SFT = OFF_W2VT + SZ_W2VT
OFF_WNF = OFF_SFT + SZ_SFT
OFF_WL = OFF_WNF + SZ_WNF
OFF_W1B = OFF_WL + SZ_WL
OFF_BIAS = OFF_W1B + SZ_W1B
SZ_IDX = 128 * 2 * IDX_COLS                  # i16 payload, bitcast as f16
OFF_IDX = OFF_BIAS + SZ_BIAS
MEGA_SZ = OFF_IDX + SZ_IDX


def build_kernel(n_cores=N_CORES, collective_mode="split", probe=None,
                 fuse_stages=True):
    n_nodes = NODE_SHARD * n_cores
    n_batches = E_CORE // BATCH
    TILE_E = 128 * KC_D * 128        # f16 elems per nfT/sfT tile

    nc = bacc.Bacc("TRN2", target_bir_lowering=False, debug=False,
                   num_devices=n_cores)

    # ---------------- I/O (consolidated: launch cost ~ #buffers) --------
    mega = nc.declare_dram_parameter("mega", [MEGA_SZ], F16, isOutput=False)
    out2 = nc.declare_dram_parameter("out2", [2, E_CORE, DOUT], F16,
                                     isOutput=True)

    # ---------------- internal DRAM ----------------
    # merged shard: rows [0:shard]=Tsrc shard, [shard:2*shard]=Tdst shard
    tbl_sh = nc.dram_tensor("tbl_shard", [2 * NODE_SHARD, TBL], F16)
    tbl_full = nc.dram_tensor("tbl_full", [2 * n_nodes, TBL], F16,
                              addr_space="Shared")

    with tile.TileContext(nc) as tc:
        with (
            tc.tile_pool(name="const", bufs=1) as cpool,
            tc.tile_pool(name="psum_b", bufs=1, space="PSUM") as pbias,
            tc.tile_pool(name="sfw", bufs=1) as sfwpool,
        ):
            # persistent constants in SBUF
            w_nf_sb = cpool.tile([128, KC_D, TBL], F16)
            nc.sync.dma_start(
                w_nf_sb[:], mega[OFF_WNF:OFF_WNF + SZ_WNF].rearrange(
                    "(c p n) -> p c n", p=128, n=TBL))
            w_l_sb = cpool.tile([128, KC_W, TBL], F16)
            nc.sync.dma_start(
                w_l_sb[:], mega[OFF_WL:OFF_WL + SZ_WL].rearrange(
                    "(c p n) -> p c n", p=128, n=TBL))
            w1b_sb = cpool.tile([128, KC_D, DOUT], F16)
            nc.sync.dma_start(
                w1b_sb[:], mega[OFF_W1B:OFF_W1B + SZ_W1B].rearrange(
                    "(c p n) -> p c n", p=128, n=DOUT))
            bias_sb = cpool.tile([1, TBL], F16)
            nc.sync.dma_start(
                bias_sb[:], mega[OFF_BIAS:OFF_BIAS + SZ_BIAS].rearrange(
                    "(a n) -> a n", a=1))
            idx_sb = cpool.tile([128, 2 * IDX_COLS], I16)
            nc.sync.dma_start(
                idx_sb[:], mega[OFF_IDX:OFF_IDX + SZ_IDX].bitcast(I16)
                .rearrange("(p n) -> p n", p=128))
            ones_sb = cpool.tile([1, 128], F16)
            nc.vector.memset(ones_sb[:], 1.0)

            # broadcast bias to all 128 partitions: psum = ones.T @ bias
            bias_full = cpool.tile([128, TBL], F32)
            pb = pbias.tile([128, TBL], F32)
            nc.tensor.matmul(pb[:], ones_sb[:], bias_sb[:], start=True, stop=True)
            nc.vector.tensor_copy(bias_full[:], pb[:])

            # stage-A result buffer: sfW[t] = (s_f @ W1b) tile t, f16
            sfw_sb = sfwpool.tile([128, EDGE_TILES, DOUT], F16)

            # ============ phase 1: node-table shard ============
            with (
                tc.tile_pool(name="p1_in", bufs=2) as p1in,
                tc.tile_pool(name="p1_out", bufs=2) as p1out,
                tc.tile_pool(name="p1_psrc", bufs=2, space="PSUM") as p1psrc,
                tc.tile_pool(name="p1_pdst", bufs=2, space="PSUM") as p1pdst,
            ):
                for nt in range(NODE_TILES):
                    r0 = nt * 128
                    nfT_t = p1in.tile([128, KC_D, 128], F16, tag="nfT")
                    o = OFF_NFT + nt * TILE_E
                    nc.sync.dma_start(
                        nfT_t[:], mega[o:o + TILE_E].rearrange(
                            "(p c f) -> p c f", p=128, f=128))
                    w2vT_t = p1in.tile([128, KC_W, 128], F16, tag="w2vT")
                    o = OFF_W2VT + nt * 128 * KC_W * 128
                    nc.sync.dma_start(
                        w2vT_t[:], mega[o:o + 128 * KC_W * 128].rearrange(
                            "(p c f) -> p c f", p=128, f=128))

                    # Tsrc = [P | Pl] + [b1|bl],  Tdst = [Q | Ql]
                    ps = p1psrc.tile([128, TBL], F32)
                    pd = p1pdst.tile([128, TBL], F32)
                    for kc in range(KC_D):
                        nc.tensor.matmul(
                            ps[:, 0:DOUT], nfT_t[:, kc, :], w_nf_sb[:, kc, 0:DOUT],
                            start=(kc == 0), stop=(kc == KC_D - 1))
                    for kc in range(KC_W):
                        nc.tensor.matmul(
                            ps[:, DOUT:TBL], w2vT_t[:, kc, :], w_l_sb[:, kc, 0:DOUT],
                            start=(kc == 0), stop=(kc == KC_W - 1))
                    for kc in range(KC_D):
                        nc.tensor.matmul(
                            pd[:, 0:DOUT], nfT_t[:, kc, :], w_nf_sb[:, kc, DOUT:TBL],
                            start=(kc == 0), stop=(kc == KC_D - 1))
                    for kc in range(KC_W):
                        nc.tensor.matmul(
                            pd[:, DOUT:TBL], w2vT_t[:, kc, :], w_l_sb[:, kc, DOUT:TBL],
                            start=(kc == 0), stop=(kc == KC_W - 1))

                    src_o = p1out.tile([128, TBL], F16, tag="src_o")
                    dst_o = p1out.tile([128, TBL], F16, tag="dst_o")
                    nc.vector.tensor_add(src_o[:], ps[:], bias_full[:])
                    nc.scalar.copy(dst_o[:], pd[:])
                    nc.sync.dma_start(tbl_sh[r0:r0 + 128, :], src_o[:])
                    nc.sync.dma_start(
                        tbl_sh[NODE_SHARD + r0:NODE_SHARD + r0 + 128, :], dst_o[:])

            # ============ AllGather the tables ============
            groups = [list(range(n_cores))]
            if collective_mode == "merged":
                # timing probe only (idx remap assumes split layout)
                nc.gpsimd.collective_compute(
                    "AllGather", mybir.AluOpType.bypass, replica_groups=groups,
                    ins=[tbl_sh[:]], outs=[tbl_full[:]])
            elif collective_mode == "split":
                # tbl_full rows [0:n_nodes] = Tsrc (node n at row n), rows
                # [n_nodes:] = Tdst (row n_nodes + n).  Two 16MiB collectives
                # measured faster than one merged 32MiB on hardware (the
                # cost model says the opposite; trust the hardware).
                nc.gpsimd.collective_compute(
                    "AllGather", mybir.AluOpType.bypass, replica_groups=groups,
                    ins=[tbl_sh[0:NODE_SHARD, :]],
                    outs=[tbl_full[0:n_nodes, :]])
                nc.gpsimd.collective_compute(
                    "AllGather", mybir.AluOpType.bypass, replica_groups=groups,
                    ins=[tbl_sh[NODE_SHARD:2 * NODE_SHARD, :]],
                    outs=[tbl_full[n_nodes:2 * n_nodes, :]])
            elif collective_mode == "none":
                pass  # timing-probe only: gathers read garbage
            else:
                raise ValueError(collective_mode)

            # ============ phase 2, stage A: sfW = s_f @ W1b ============
            # (independent of the collective; overlaps it)
            stage_a_tiles = 0 if probe == "noA" else EDGE_TILES
            from contextlib import ExitStack
            stack = ExitStack()
            if True:
                p2sft = stack.enter_context(tc.tile_pool(name="p2_sfT", bufs=4))
                p2pa = stack.enter_context(
                    tc.tile_pool(name="p2_pa", bufs=6, space="PSUM"))
                for t0 in range(0, stage_a_tiles, 2):
                    sfT_t = p2sft.tile([128, 2, KC_D, 128], F16, tag="sfT")
                    o = OFF_SFT + t0 * TILE_E
                    nc.sync.dma_start(
                        sfT_t[:], mega[o:o + 2 * TILE_E].rearrange(
                            "(t p c f) -> p t c f", t=2, p=128, f=128))
                    pa = p2pa.tile([128, 2, DOUT], F32)
                    for tt in range(2):
                        for kc in range(KC_D):
                            nc.tensor.matmul(
                                pa[:, tt, :], sfT_t[:, tt, kc, :], w1b_sb[:, kc, :],
                                start=(kc == 0), stop=(kc == KC_D - 1))
                    nc.scalar.copy(sfw_sb[:, t0:t0 + 2, :], pa[:])

            # ============ phase 2, stage B: gather + add + relu + out =====
            # By default opened AFTER the stage-A pools close: the stack
            # allocator reuses stage A's SBUF zone, so every stage-B tile
            # carries a dependency on stage-A completion, which keeps the
            # scheduler from interleaving stage-B ops (which stall on the
            # collective) ahead of stage-A ops on shared engine queues.
            # fuse_stages=True keeps the stage-A pools open instead.
            if not fuse_stages:
                stack.close()
            GRP = 4                       # edge tiles per stage-B group
            stage_b_batches = 0 if probe == "noB" else n_batches
            with (
                tc.tile_pool(name="p2_g", bufs=4) as p2g,
                tc.tile_pool(name="p2_sum", bufs=4) as p2sum,
                tc.tile_pool(name="p2_out", bufs=4) as p2out,
            ):
                if probe == "noB":
                    dummy = p2out.tile([128, GRP, DOUT], F16, tag="oe")
                    nc.scalar.copy(dummy[:], sfw_sb[:, 0:GRP, :])
                    nc.scalar.dma_start(
                        out2[0, 0:GRP * 128, :].rearrange(
                            "(t p) n -> p t n", p=128), dummy[:])
                for b in range(stage_b_batches):
                    # batch idx stream: cols [c0:c0+64] are the 1024 src
                    # rows, [c0+64:c0+128] the 1024 dst rows.  One gather
                    # per 1024 rows: dma_gather descriptors must fit the
                    # 1024-entry SWDGE scratch (2048-row gathers fault).
                    c0 = b * (2 * BATCH // 16)
                    g_src = p2g.tile([128, BATCH_TILES, TBL], F16, tag="gs")
                    nc.gpsimd.dma_gather(
                        g_src[:], tbl_full[:], idx_sb[:, c0:c0 + BATCH // 16],
                        BATCH, BATCH, TBL)
                    g_dst = p2g.tile([128, BATCH_TILES, TBL], F16, tag="gd")
                    nc.gpsimd.dma_gather(
                        g_dst[:], tbl_full[:],
                        idx_sb[:, c0 + BATCH // 16:c0 + 2 * BATCH // 16],
                        BATCH, BATCH, TBL)

                    for t in range(0, BATCH_TILES, GRP):
                        at = b * BATCH_TILES + t
                        e0 = at * EDGE_TILE
                        tsum = p2sum.tile([128, GRP, TBL], F16, tag="tsum")
                        nc.vector.tensor_add(tsum[:], g_src[:, t:t + GRP, :],
                                             g_dst[:, t:t + GRP, :])
                        oe = p2out.tile([128, GRP, DOUT], F16, tag="oe")
                        ol = p2out.tile([128, GRP, DOUT], F16, tag="ol")
                        esum = p2sum.tile([128, GRP, DOUT], F16, tag="esum")
                        sfw_in = (tsum[:, :, 0:DOUT] if probe == "noA"
                                  else sfw_sb[:, at:at + GRP, :])
                        nc.vector.tensor_add(esum[:], tsum[:, :, 0:DOUT],
                                             sfw_in)
                        nc.scalar.activation(
                            oe[:], esum[:], mybir.ActivationFunctionType.Relu)
                        nc.vector.tensor_scalar_max(
                            ol[:], tsum[:, :, DOUT:TBL], 0.0)
                        nc.scalar.dma_start(
                            out2[0, e0:e0 + GRP * 128, :].rearrange(
                                "(t p) n -> p t n", p=128), oe[:])
                        nc.sync.dma_start(
                            out2[1, e0:e0 + GRP * 128, :].rearrange(
                                "(t p) n -> p t n", p=128), ol[:])

            if fuse_stages:
                stack.close()

    nc.compile()
    return nc


# ---------------------------------------------------------------- host side
def _wrap_idx(ix, batch):
    """int16 index layout for dma_gather: idx j of a batch sits at
    (partition j%16, column j//16); 16-row block replicated to 128."""
    e = ix.shape[0]
    n_b = e // batch
    cols = batch // 16
    arr = np.zeros((16, e // 16), dtype=np.int16)
    for b in range(n_b):
        blk = ix[b * batch:(b + 1) * batch].astype(np.int16).reshape(cols, 16).T
        arr[:, b * cols:(b + 1) * cols] = blk
    return np.ascontiguousarray(np.tile(arr, (8, 1)))


def _to_lhsT(x16, kc):
    """[rows, kc*128] f16 -> [rows/128 tiles, 128 K-partitions, kc*128] lhsT."""
    tiles = x16.shape[0] // 128
    return np.ascontiguousarray(
        x16.reshape(tiles, 128, kc, 128).transpose(0, 3, 2, 1)
        .reshape(tiles, 128, kc * 128))


_NC_CACHE = {}


def make_in_maps(n_f, word2vec, s_f, W1, b1, Wl, bl, src, dst):
    n_f = np.asarray(n_f, dtype=np.float32)
    word2vec = np.asarray(word2vec, dtype=np.float32)
    s_f = np.asarray(s_f, dtype=np.float32)
    W1 = np.asarray(W1, dtype=np.float32)
    Wl = np.asarray(Wl, dtype=np.float32)
    b1 = np.asarray(b1, dtype=np.float32)
    bl = np.asarray(bl, dtype=np.float32)
    src = np.asarray(src).astype(np.int64)
    dst = np.asarray(dst).astype(np.int64)

    w2v_pad = np.zeros((N_NODES, DW_PAD), np.float16)
    w2v_pad[:, :300] = word2vec.astype(np.float16)
    n_f16 = n_f.astype(np.float16)
    s_f16 = s_f.astype(np.float16)

    w_nf = np.ascontiguousarray(
        np.concatenate([W1[0:D], W1[2 * D:3 * D]], axis=1)).astype(np.float16)
    w_l = np.zeros((DW_PAD, TBL), np.float16)
    w_l[:300, 0:DOUT] = Wl[0:300]
    w_l[:300, DOUT:TBL] = Wl[300:600]
    w1b = np.ascontiguousarray(W1[D:2 * D]).astype(np.float16)
    bias_src = np.concatenate([b1, bl]).astype(np.float16)
    w_tail = np.concatenate(
        [w_nf.ravel(), w_l.ravel(), w1b.ravel(), bias_src])

    # gather rows in the split-AllGather table layout:
    #   Tsrc occupies rows [0:N_NODES] (node n at row n), Tdst rows
    #   [N_NODES:2*N_NODES] (node n at row N_NODES + n)
    row_src = src
    row_dst = N_NODES + dst
    # per 1024-edge batch, the gather fetches 2048 rows: first the 1024
    # src rows (chunks 0-7 of the gather output), then the 1024 dst rows
    # (chunks 8-15)

    in_maps = []
    for k in range(N_CORES):
        es, ee = k * E_CORE, (k + 1) * E_CORE
        ns, ne = k * NODE_SHARD, (k + 1) * NODE_SHARD
        mega = np.empty(MEGA_SZ, np.float16)
        mega[OFF_NFT:OFF_NFT + SZ_NFT] = _to_lhsT(n_f16[ns:ne], KC_D).ravel()
        mega[OFF_W2VT:OFF_W2VT + SZ_W2VT] = \
            _to_lhsT(w2v_pad[ns:ne], KC_W).ravel()
        mega[OFF_SFT:OFF_SFT + SZ_SFT] = _to_lhsT(s_f16[es:ee], KC_D).ravel()
        mega[OFF_WNF:OFF_WNF + SZ_WNF + SZ_WL + SZ_W1B + SZ_BIAS] = w_tail
        rs = row_src[es:ee].reshape(-1, BATCH)
        rd = row_dst[es:ee].reshape(-1, BATCH)
        stream = np.concatenate([rs, rd], axis=1).ravel()
        idx = _wrap_idx(stream, 2 * BATCH)
        mega[OFF_IDX:] = np.ascontiguousarray(idx).ravel().view(np.float16)
        in_maps.append({"mega": mega})

    return in_maps


def kernel(n_f, word2vec, s_f, W1, b1, Wl, bl, src, dst):
    if "nc" not in _NC_CACHE:
        _NC_CACHE["nc"] = build_kernel()
    nc = _NC_CACHE["nc"]
    in_maps = make_in_maps(n_f, word2vec, s_f, W1, b1, Wl, bl, src, dst)
    res = run_bass_kernel_spmd(nc, in_maps, list(range(N_CORES)))
    _NC_CACHE["last_results"] = res
    e_f = np.concatenate(
        [res.results[k]["out2"][0] for k in range(N_CORES)]).astype(np.float32)
    e_f_lang = np.concatenate(
        [res.results[k]["out2"][1] for k in range(N_CORES)]).astype(np.float32)
    return (e_f, e_f_lang)
